# revision 1
# baseline (speedup 1.0000x reference)
"""GQA attention kernel for Trainium2 (Bass/Tile), 8-core SPMD.

Problem: B=2, N=2048, DIM=1024, 16 query heads / 4 KV heads, head_dim=64, fp32.
Sharding: core c = (batch b=c//4, kv-group g=c%4). Each core computes its
group's 4 query heads + 1 shared KV head over the full sequence, and a partial
output projection (its 256 rows of Wo). Host sums the 4 group partials per
batch and adds the bias.

Per-core layout (all "T" tensors keep head_dim/feature on partitions, seq on
free dim):
  xT   [128, N] x 8     : x^T, from PE transposes of DMA'd x tiles
  qt_p [128, N] x 2     : Q^T head pairs (head 2p on partitions 0-63, 2p+1 on 64-127)
  kkT  [128, N]         : K^T duplicated (rows 0-63 == 64-127) to feed row-paired
                          score matmuls for both heads of a pair
  vn   [128, 16, 64] bf16 : V in normal layout (seq on partitions), for P@V
Scores are computed transposed (S^T tile [128 keys, 512 queries]) so softmax
needs no max-subtraction (scores bounded ~|8|) and exp output P^T feeds P@V
directly.  Sum-of-exp per query rides on 4-way column-tiled ones-matmuls.
"""

import sys

if "/opt/trn_rl_repo" not in sys.path:
    sys.path.insert(0, "/opt/trn_rl_repo")

from contextlib import ExitStack

import numpy as np

import concourse.bass as bass
import concourse.mybir as mybir
import concourse.tile as tile
from concourse import bacc, bass_utils
from concourse.bass import ds, ts
from concourse.masks import make_identity

F32 = mybir.dt.float32
F32R = mybir.dt.float32r
BF16 = mybir.dt.bfloat16
EXPF = mybir.ActivationFunctionType.Exp

DIM = 1024
D = 64  # head dim
SCALE = D ** -0.5


def build_nc(NSEQ=2048):
    KT = NSEQ // 128   # key tiles
    QC = NSEQ // 512   # query chunks of 512
    DKT = DIM // 128   # contraction tiles for projections

    nc = bacc.Bacc("TRN2", target_bir_lowering=False, debug=False)
    x = nc.dram_tensor("x", [NSEQ, DIM], F32, kind="ExternalInput").ap()
    wq = nc.dram_tensor("wq", [DIM, 256], F32, kind="ExternalInput").ap()
    wk = nc.dram_tensor("wk", [DIM, D], F32, kind="ExternalInput").ap()
    wv = nc.dram_tensor("wv", [DIM, D], F32, kind="ExternalInput").ap()
    wo = nc.dram_tensor("wo", [256, DIM], F32, kind="ExternalInput").ap()
    out = nc.dram_tensor("out", [DIM, NSEQ], F32, kind="ExternalOutput").ap()
    scr = nc.dram_tensor("scr", [QC, 4, 512], F32, kind="Internal").ap()

    with tile.TileContext(nc) as tc, ExitStack() as ctx:
        sb = ctx.enter_context(tc.tile_pool(name="sb", bufs=1))

        wq_sb = sb.tile([128, DKT, 256], F32R)
        wkk_sb = sb.tile([128, DKT, 128], F32R)
        wv_sb = sb.tile([128, DKT, D], F32R)
        wo_sb = sb.tile([128, 2, DIM], F32R)
        ident = sb.tile([128, 128], F32)
        ones_k = sb.tile([128, 1], BF16)
        warm = sb.tile([128, 1], F32)

        nc.sync.dma_start(out=wq_sb, in_=wq.rearrange("(t p) m -> p t m", p=128).bitcast(F32R))
        nc.sync.dma_start(out=wkk_sb[:, :, 0:D], in_=wk.rearrange("(t p) m -> p t m", p=128).bitcast(F32R))
        nc.sync.dma_start(out=wkk_sb[:, :, D:128], in_=wk.rearrange("(t p) m -> p t m", p=128).bitcast(F32R))
        nc.sync.dma_start(out=wv_sb, in_=wv.rearrange("(t p) m -> p t m", p=128).bitcast(F32R))
        nc.sync.dma_start(out=wo_sb, in_=wo.rearrange("(t p) m -> p t m", p=128).bitcast(F32R))
        make_identity(nc, ident)
        nc.vector.memset(ones_k, 1.0)
        # preload the exp table set off the critical path
        nc.scalar.activation(out=warm, in_=ones_k, func=EXPF, scale=1.0)

        xT = [sb.tile([128, NSEQ], F32R, name=f"xT{d}") for d in range(DKT)]
        qt = [sb.tile([128, NSEQ], F32R, name=f"qt{p}") for p in range(2)]
        kkT = sb.tile([128, NSEQ], F32R)
        vT = sb.tile([64, NSEQ], F32)
        vn1 = sb.tile([128, KT, D + 1], BF16)
        nc.vector.memset(vn1, 1.0)
        aout = [sb.tile([128, NSEQ], F32R, name=f"aout{p}") for p in range(2)]

        # stage pools: xpool only (psum unified with attention pools below)
        xpool = ctx.enter_context(tc.tile_pool(name="xp", bufs=5))

        # ---------------- stage 2: attention loop ----------------
        ps_s = ctx.enter_context(tc.tile_pool(name="ps_s", bufs=2, space="PSUM"))
        ps_pv = ctx.enter_context(tc.tile_pool(name="ps_pv", bufs=4, space="PSUM"))

        def emit_sgroup(sg):
            """Load+transpose x chunk sg; project K/V for that key chunk."""
            xs = [xpool.tile([128, DIM], F32, tag="xs", name=f"xs{sg}_{_i}") for _i in range(4)]
            for i in range(4):
                nc.sync.dma_start(out=xs[i], in_=x[ts(sg * 4 + i, 128), :])
            for d in range(DKT):
                ptr = ps_s.tile([128, 1024], F32, tag="sc", name=f"ptr{sg}_{d}")
                for i in range(4):
                    nc.tensor.transpose(ptr[:, ts(i, 128)], xs[i][:, ts(d, 128)], ident)
                nc.vector.tensor_copy(xT[d][:, ds(sg * 512, 512)], ptr[:, 0:512])
            pk = ps_s.tile([128, 1024], F32, tag="sc", name=f"pk{sg}")
            for d in range(DKT):
                nc.tensor.matmul(pk[:, 0:512], wkk_sb[:, d, :], xT[d][:, ds(sg * 512, 512)],
                                 start=(d == 0), stop=(d == DKT - 1))
            nc.vector.tensor_copy(kkT[:, ds(sg * 512, 512)], pk[:, 0:512])
            pv_ = ps_s.tile([128, 1024], F32, tag="sc", name=f"pvp{sg}")
            for d in range(DKT):
                nc.tensor.matmul(pv_[0:64, 0:512], wv_sb[:, d, :], xT[d][:, ds(sg * 512, 512)],
                                 start=(d == 0), stop=(d == DKT - 1))
            nc.vector.tensor_copy(vT[:, ds(sg * 512, 512)], pv_[0:64, 0:512])
            ptv = ps_s.tile([128, 1024], F32, tag="sc", name=f"ptv{sg}")
            for i in range(4):
                t = sg * 4 + i
                nc.tensor.transpose(ptv[:, ds(i * D, D)], vT[:, ts(t, 128)], ident[0:64, 0:64])
            nc.vector.tensor_copy(vn1[:, sg * 4:(sg + 1) * 4, 0:D], ptv[:, 0:4 * D])

        def emit_qt(qc):
            for p in range(2):
                pq = ps_s.tile([128, 1024], F32, tag="sc", name=f"pq{qc}_{p}")
                for d in range(DKT):
                    nc.tensor.matmul(pq[:, 0:512], wq_sb[:, d, ts(p, 128)], xT[d][:, ds(qc * 512, 512)],
                                     start=(d == 0), stop=(d == DKT - 1))
                nc.vector.tensor_copy(qt[p][:, ds(qc * 512, 512)], pq[:, 0:512])
        ptp = ctx.enter_context(tc.tile_pool(name="ptp", bufs=10))
        rrp = ctx.enter_context(tc.tile_pool(name="rrp", bufs=2))
        Rp_pool = ctx.enter_context(tc.tile_pool(name="Rp", bufs=4))
        outp = ctx.enter_context(tc.tile_pool(name="outp", bufs=3))

        state = {}

        def emit_norm(qc):
            pvs = state[qc]
            rr = rrp.tile([128, 2048], F32, tag="rr")
            for h in range(4):
                nc.vector.reciprocal(out=rr[ds(64, 1), ds(h * 512, 512)], in_=pvs[h][ds(64, 1), :])
            r64 = rr[ds(64, 1), :]
            nc.sync.dma_start(
                out=scr[qc:qc + 1, :, :],
                in_=bass.AP(tensor=r64.tensor, offset=r64.offset,
                            ap=[[r64.ap[0][0], 1], [512, 4], [1, 512]]),
            )
            for p in range(2):
                Rt = Rp_pool.tile([128, 512], F32, tag="R")
                for i in range(2):
                    src = bass.AP(tensor=scr.tensor,
                                  offset=scr.offset + (qc * 4 + 2 * p + i) * 512,
                                  ap=[[0, 64], [1, 512]])
                    nc.sync.dma_start(out=Rt[ds(i * 64, 64), :], in_=src)
                for i in range(2):
                    nc.vector.tensor_mul(aout[p][ds(i * 64, 64), ds(qc * 512, 512)],
                                         pvs[2 * p + i][0:64, :], Rt[ds(i * 64, 64), :])

        def emit_outproj(qc):
            for od in range(DIM // 128):
                po = ps_s.tile([128, 512], F32, tag="sc")
                nc.tensor.matmul(po, wo_sb[:, 0, ts(od, 128)], aout[0][:, ds(qc * 512, 512)],
                                 start=True, stop=False)
                nc.tensor.matmul(po, wo_sb[:, 1, ts(od, 128)], aout[1][:, ds(qc * 512, 512)],
                                 start=False, stop=True)
                ot = outp.tile([128, 512], F32, tag="ot")
                nc.vector.tensor_copy(ot, po)
                nc.sync.dma_start(out=out[ts(od, 128), ds(qc * 512, 512)], in_=ot)

        pending_pv = []

        def flush_pv():
            for (qc_, j_, h_, pt_) in pending_pv:
                for t in range(2):
                    kt = 2 * j_ + t
                    nc.tensor.matmul(state[qc_][h_][0:65, :],
                                     vn1[:, kt, :], pt_[:, ds(t * 512, 512)],
                                     start=(kt == 0), stop=(kt == KT - 1))
            pending_pv.clear()

        def emit_quanta(qc, j):
            new_pv = []
            for h in range(4):
                p, i = h // 2, h % 2
                psc = ps_s.tile([128, 1024], F32, tag="sc", name=f"psc{qc}_{j}_{h}")
                for t in range(2):
                    kt = 2 * j + t
                    nc.tensor.matmul(psc[:, ds(t * 512, 512)],
                                     kkT[ds(i * 64, 64), ts(kt, 128)],
                                     qt[p][ds(i * 64, 64), ds(qc * 512, 512)],
                                     start=True, stop=True)
                pt = ptp.tile([128, 1024], BF16, tag="pt", name=f"pt{qc}_{j}_{h}")
                nc.scalar.activation(out=pt, in_=psc, func=EXPF, scale=SCALE)
                new_pv.append((qc, j, h, pt))
            flush_pv()
            pending_pv.extend(new_pv)

        # interleaved prologue: per key chunk, project K/V then run qc=0 attention on it
        state[0] = [ps_pv.tile([128, 512], F32, tag="pv", name=f"pv0_{h}") for h in range(4)]
        for sg in range(QC):
            emit_sgroup(sg)
            if sg == 0:
                emit_qt(0)
            emit_quanta(0, 2 * sg)
            emit_quanta(0, 2 * sg + 1)
        flush_pv()
        emit_norm(0)
        for qc in range(1, QC):
            pvs = [ps_pv.tile([128, 512], F32, tag="pv", name=f"pv{qc}_{h}") for h in range(4)]
            state[qc] = pvs
            emit_qt(qc)
            for j in range(KT // 2):
                emit_quanta(qc, j)
                if j == 1:
                    emit_outproj(qc - 1)
            flush_pv()
            emit_norm(qc)
        emit_outproj(QC - 1)

    nc.compile()
    return nc


_CACHE = {}


def _get_nc(NSEQ):
    if NSEQ not in _CACHE:
        _CACHE[NSEQ] = build_nc(NSEQ)
    return _CACHE[NSEQ]


def kernel(x, Wq, Wk, Wv, Wo, bo):
    """Full-input entry point: shard over 8 cores, run, gather."""
    x, Wq, Wk, Wv, Wo, bo = (np.asarray(a, np.float32) for a in (x, Wq, Wk, Wv, Wo, bo))
    B, N, C = x.shape
    nc = _get_nc(N)
    in_maps = []
    for c in range(8):
        b, g = c // 4, c % 4
        in_maps.append({
            "x": np.ascontiguousarray(x[b]),
            "wq": np.ascontiguousarray(Wq[:, g * 256:(g + 1) * 256]),
            "wk": np.ascontiguousarray(Wk[:, g * D:(g + 1) * D]),
            "wv": np.ascontiguousarray(Wv[:, g * D:(g + 1) * D]),
            "wo": np.ascontiguousarray(Wo[g * 256:(g + 1) * 256, :]),
        })
    res = bass_utils.run_bass_kernel_spmd(nc, in_maps, core_ids=list(range(8)))
    outs = [res.results[c]["out"] for c in range(8)]
    full = np.empty((B, N, C), np.float32)
    for b in range(B):
        acc = outs[4 * b].astype(np.float32)
        for g in range(1, 4):
            acc = acc + outs[4 * b + g]
        full[b] = acc.T + bo[None, :]
    return full



# revision 7
# speedup vs baseline: 1.3880x; 1.3880x over previous
"""GQA attention kernel for Trainium2 (Bass/Tile), 8-core SPMD.

Problem: B=2, N=2048, DIM=1024, 16 query heads / 4 KV heads, head_dim=64, fp32.
Sharding: core c = (batch b=c//4, kv-group g=c%4). Each core computes its
group's 4 query heads + 1 shared KV head over the full sequence, and a partial
output projection (its 256 rows of Wo). Host sums the 4 group partials per
batch and adds the bias.

Data path (all bf16 on PE, fp32 in PSUM):
  - Host pre-transposes x -> xT [1024, 2048] bf16 per batch, slices weights.
  - Projections: Q^T [256(2 head pairs), N], K^T [64, N] (duplicated onto
    both partition halves for head-pair score matmuls), V natural [N, 64]
    stored as vones [128, 16, 65] with an all-ones column for row-sums.
  - Scores S^T [128 keys, q] = K^T-tile (stationary) x Q^T (moving); exp on
    ACT into P [keys, q] bf16 (no max subtraction; |s| small by construction).
  - PV: P-tile [keys, 128 q] stationary x vones [keys, 65] moving ->
    psum [128 q, 65] accumulated over 16 key tiles; col 64 = sum of exp.
  - Normalize with per-partition scalar multiply (DVE), pack A [tok, 256],
    PE-transpose to A^T, out-proj out^T[1024, N] = Wo-tile x A^T.

The q dimension is processed in two mega-chunks of 1024 so exp units are
[128,1024] (2 psum banks, double buffered = 4) + 2 PV banks + 2 outproj/
transpose banks = 8 banks total.
"""

import sys

if "/opt/trn_rl_repo" not in sys.path:
    sys.path.insert(0, "/opt/trn_rl_repo")

from contextlib import ExitStack

import numpy as np

import concourse.bass as bass
import concourse.mybir as mybir
import concourse.tile as tile
from concourse import bacc, bass_utils
from concourse.bass import ds, ts
from concourse.masks import make_identity

F32 = mybir.dt.float32
BF16 = mybir.dt.bfloat16
EXPF = mybir.ActivationFunctionType.Exp

DIM = 1024
D = 64  # head dim
SCALE = D ** -0.5


def build_nc(NSEQ=2048):
    KB = NSEQ // 128          # key tiles
    NC_CH = NSEQ // 512       # 512-token chunks
    MCS = [(0, NSEQ // 2), (NSEQ // 2, NSEQ // 2)]  # (q0, qw) mega-chunks

    nc = bacc.Bacc("TRN2", target_bir_lowering=False, debug=False)
    xT = nc.dram_tensor("xT", [DIM, NSEQ], BF16, kind="ExternalInput").ap()
    wq = nc.dram_tensor("wq", [DIM, 256], BF16, kind="ExternalInput").ap()
    wk = nc.dram_tensor("wk", [DIM, D], BF16, kind="ExternalInput").ap()
    wv = nc.dram_tensor("wv", [DIM, D], BF16, kind="ExternalInput").ap()
    wo = nc.dram_tensor("wo", [256, DIM], BF16, kind="ExternalInput").ap()
    out = nc.dram_tensor("out", [DIM, NSEQ], F32, kind="ExternalOutput").ap()

    with tile.TileContext(nc) as tc, ExitStack() as ctx:
        sb = ctx.enter_context(tc.tile_pool(name="sb", bufs=1))
        dyn = ctx.enter_context(tc.tile_pool(name="dyn", bufs=1))
        ps = ctx.enter_context(tc.tile_pool(name="ps", bufs=1, space="PSUM"))

        # ---- persistent SBUF ----
        wq_sb = sb.tile([128, 8, 256], BF16)
        wk_sb = sb.tile([128, 8, D], BF16)
        wv_sb = sb.tile([128, 8, D], BF16)
        wo_sb = sb.tile([128, 2, DIM], BF16)
        ident = sb.tile([128, 128], BF16)
        xt_sb = sb.tile([128, 8, NSEQ], BF16)
        QT = sb.tile([128, 2, NSEQ], BF16)
        KT = sb.tile([128, NSEQ], BF16)
        vones = sb.tile([128, KB, D + 1], BF16)
        AT = sb.tile([128, 2, NSEQ], BF16)
        warm = sb.tile([128, 1], F32)
        wone = sb.tile([128, 1], BF16)

        # ---- weight / x DMAs (emission order = DMA device order) ----
        nc.sync.dma_start(out=wq_sb, in_=wq.rearrange("(t p) m -> p t m", p=128))
        nc.sync.dma_start(out=xt_sb[:, :, 0:512],
                          in_=xT.rearrange("(t p) m -> p t m", p=128)[:, :, 0:512])
        nc.sync.dma_start(out=wk_sb, in_=wk.rearrange("(t p) m -> p t m", p=128))
        nc.sync.dma_start(out=wv_sb, in_=wv.rearrange("(t p) m -> p t m", p=128))
        for c in range(1, NC_CH):
            nc.sync.dma_start(out=xt_sb[:, :, ts(c, 512)],
                              in_=xT.rearrange("(t p) m -> p t m", p=128)[:, :, ts(c, 512)])
        nc.sync.dma_start(out=wo_sb, in_=wo.rearrange("(t p) m -> p t m", p=128))
        make_identity(nc, ident)
        nc.vector.memset(vones, 1.0)
        nc.vector.memset(wone, 1.0)
        nc.scalar.activation(out=warm, in_=wone, func=EXPF, scale=1.0)

        # keep PE continuously busy from t~1us so real matmuls start at the
        # ramped clock (p-state) instead of mid speed
        for w in range(24):
            pw = ps.tile([128, 512], BF16, tag="op", bufs=2, name=f"pwarm{w}")
            nc.tensor.transpose(pw[:, 0:128], ident, ident)

        # ---- prologue emitters ----
        def emit_qproj(c, p):
            """Q^T head pair p for token chunk c -> QT[:, p, c*512:+512]."""
            pq = ps.tile([128, 1024], F32, tag="sp", bufs=2, name=f"pq{c}_{p}")
            for t in range(8):
                nc.tensor.matmul(pq[:, 0:512], wq_sb[:, t, ts(p, 128)],
                                 xt_sb[:, t, ts(c, 512)],
                                 start=(t == 0), stop=(t == 7))
            nc.vector.tensor_copy(QT[:, p, ts(c, 512)], pq[:, 0:512])

        def emit_kproj(c):
            """K^T for token chunk c -> KT[0:64, c*512:+512]."""
            pk = ps.tile([128, 512], F32, tag="pv", bufs=2, name=f"pk{c}")
            for t in range(8):
                nc.tensor.matmul(pk[0:64, 0:512], wk_sb[:, t, :],
                                 xt_sb[:, t, ts(c, 512)],
                                 start=(t == 0), stop=(t == 7))
            nc.vector.tensor_copy(KT[0:64, ts(c, 512)], pk[0:64, 0:512])

        def emit_vproj(c):
            """V natural for chunk c -> vones[:, c*4:(c+1)*4, 0:64]."""
            pv = ps.tile([128, 512], F32, tag="op", bufs=2, name=f"pvv{c}")
            for tbl in range(4):
                tb = c * 4 + tbl
                for t in range(8):
                    nc.tensor.matmul(pv[:, ds(tbl * D, D)],
                                     xt_sb[:, t, ts(tb, 128)], wv_sb[:, t, :],
                                     start=(t == 0), stop=(t == 7))
            nc.vector.tensor_copy(
                vones[:, ds(c * 4, 4), 0:D],
                pv[:, 0:4 * D].rearrange("p (t d) -> p t d", d=D))

        def emit_ktdup():
            # duplicate K^T onto partitions 64:128 (SBUF->SBUF DMA crosses
            # partitions; engines cannot)
            nc.sync.dma_start(out=KT[64:128, :], in_=KT[0:64, :])

        # ---- attention state ----
        P_of = {}      # window -> P tile
        A_of = {}      # mci -> A tile

        def emit_scores_exp(mci, h, kb):
            q0, qw = MCS[mci]
            p, i = h // 2, h % 2
            sp = ps.tile([128, 1024], F32, tag="sp", bufs=2,
                         name=f"sp{mci}_{h}_{kb}")
            for j in range(qw // 512):
                nc.tensor.matmul(sp[:, ts(j, 512)],
                                 KT[ds(i * D, D), ts(kb, 128)],
                                 QT[ds(i * D, D), p, ds(q0 + j * 512, 512)],
                                 start=True, stop=True)
            nc.scalar.activation(out=P_of[(mci, h)][:, kb, 0:qw],
                                 in_=sp[:, 0:qw], func=EXPF, scale=SCALE)

        def emit_pv(mci, h, qb):
            """PV for query block qb (128 tokens) of head h, mega-chunk mci;
            includes normalize into A and (for odd heads) the A^T transpose."""
            q0, qw = MCS[mci]
            P = P_of[(mci, h)]
            acc = ps.tile([128, 512], F32, tag="pv", bufs=2,
                          name=f"acc{mci}_{h}_{qb}")
            for kb in range(KB):
                nc.tensor.matmul(acc[:, 0:D + 1], P[:, kb, ds(qb * 128, 128)],
                                 vones[:, kb, :],
                                 start=(kb == 0), stop=(kb == KB - 1))
            rc = dyn.tile([128, 1], F32, tag="rc", bufs=6,
                          name=f"rc{mci}_{h}_{qb}")
            nc.vector.reciprocal(out=rc, in_=acc[:, D:D + 1])
            nc.vector.tensor_scalar_mul(
                A_of[mci][:, qb, ds(h * D, D)], acc[:, 0:D], rc)
            if h % 2 == 1:
                pr = h // 2
                tp = ps.tile([128, 512], BF16, tag="op", bufs=2,
                             name=f"tp{mci}_{h}_{qb}")
                nc.tensor.transpose(tp[:, 0:128], A_of[mci][:, qb, ts(pr, 128)],
                                    ident)
                nc.vector.tensor_copy(
                    AT[:, pr, ds(q0 + qb * 128, 128)], tp[:, 0:128])

        def emit_outproj(mci, ct, tch, act_copy=False):
            """out^T[ct*128:+128, q0+tch*512:+512]."""
            q0, qw = MCS[mci]
            po = ps.tile([128, 512], F32, tag="op", bufs=2,
                         name=f"po{mci}_{ct}_{tch}")
            for ft in range(2):
                nc.tensor.matmul(po, wo_sb[:, ft, ts(ct, 128)],
                                 AT[:, ft, ds(q0 + tch * 512, 512)],
                                 start=(ft == 0), stop=(ft == 1))
            ot = dyn.tile([128, 512], F32, tag="os", bufs=3,
                          name=f"ot{mci}_{ct}_{tch}")
            if act_copy:
                nc.scalar.copy(ot, po)
            else:
                nc.vector.tensor_copy(ot, po)
            nc.sync.dma_start(out=out[ts(ct, 128), ds(q0 + tch * 512, 512)],
                              in_=ot)

        # ---- window schedule with sprinkled side-work ----
        windows = [(mci, h) for mci in range(2) for h in range(4)]

        # pre-window prologue: enough to start (0, h0): Q pair0 chunks 0..1,
        # K chunk 0
        emit_qproj(0, 0)
        emit_kproj(0)
        emit_qproj(1, 0)

        def sprinkles(wi):
            mci, h = windows[wi]
            th = []
            if wi == 0:
                th += [lambda c=c: emit_kproj(c) for c in (1, 2, 3)]
                th += [lambda c=c: emit_vproj(c) for c in range(4)]
                th.append(emit_ktdup)
                th += [lambda: emit_qproj(0, 1), lambda: emit_qproj(1, 1),
                       lambda: emit_qproj(2, 0)]
            else:
                pmci, ph = windows[wi - 1]
                nqb = MCS[pmci][1] // 128
                th += [lambda qb=qb, m=pmci, hh=ph: emit_pv(m, hh, qb)
                       for qb in range(nqb)]
                if wi == 1:
                    th += [lambda: emit_qproj(3, 0), lambda: emit_qproj(2, 1),
                           lambda: emit_qproj(3, 1)]
                if wi in (5, 6):  # out-proj of mc0 during (1,h1) and (1,h2)
                    units = [(ct, tch) for tch in range(2) for ct in range(8)]
                    half = units[:8] if wi == 5 else units[8:]
                    th += [lambda u=u: emit_outproj(0, u[0], u[1])
                           for u in half]
            return th

        for wi, (mci, h) in enumerate(windows):
            P_of[(mci, h)] = dyn.tile([128, KB, 1024], BF16, tag="P", bufs=2,
                                      name=f"P{mci}_{h}")
            if h == 1:
                A_of[mci] = dyn.tile([128, MCS[mci][1] // 128, 256], BF16,
                                     tag="A", bufs=2, name=f"A{mci}")
            th = sprinkles(wi)
            for kb in range(KB):
                emit_scores_exp(mci, h, kb)
                # spread side work across the window
                while th and len(th) > (KB - 1 - kb) * (len(th) + KB - 1) // KB:
                    th.pop(0)()
            while th:
                th.pop(0)()

        # ---- tail: PV of last head + out-proj of mc1 ----
        for qb in range(MCS[1][1] // 128):
            emit_pv(1, 3, qb)
        for ct in range(8):
            for tch in range(2):
                emit_outproj(1, ct, tch, act_copy=(ct + tch) % 2 == 0)

    nc.compile()
    return nc


_CACHE = {}


def _get_nc(NSEQ):
    if NSEQ not in _CACHE:
        _CACHE[NSEQ] = build_nc(NSEQ)
    return _CACHE[NSEQ]


def kernel(x, Wq, Wk, Wv, Wo, bo):
    """Full-input entry point: shard over 8 cores, run, gather."""
    import ml_dtypes
    bf16 = ml_dtypes.bfloat16
    x = np.asarray(x, np.float32)
    Wq = np.asarray(Wq, np.float32)
    Wk = np.asarray(Wk, np.float32)
    Wv = np.asarray(Wv, np.float32)
    Wo = np.asarray(Wo, np.float32)
    bo = np.asarray(bo, np.float32)
    B, N, C = x.shape
    nc = _get_nc(N)
    in_maps = []
    for c in range(8):
        b, g = c // 4, c % 4
        in_maps.append({
            "xT": np.ascontiguousarray(x[b].T).astype(bf16),
            "wq": np.ascontiguousarray(Wq[:, g * 256:(g + 1) * 256]).astype(bf16),
            "wk": np.ascontiguousarray(Wk[:, g * D:(g + 1) * D]).astype(bf16),
            "wv": np.ascontiguousarray(Wv[:, g * D:(g + 1) * D]).astype(bf16),
            "wo": np.ascontiguousarray(Wo[g * 256:(g + 1) * 256, :]).astype(bf16),
        })
    res = bass_utils.run_bass_kernel_spmd(nc, in_maps, core_ids=list(range(8)))
    outs = [res.results[c]["out"] for c in range(8)]
    full = np.empty((B, N, C), np.float32)
    for b in range(B):
        acc = outs[4 * b].astype(np.float32)
        for g in range(1, 4):
            acc = acc + outs[4 * b + g]
        full[b] = acc.T + bo[None, :]
    return full


# revision 13
# speedup vs baseline: 1.4599x; 1.0519x over previous
"""GQA attention kernel for Trainium2 (Bass/Tile), 8-core SPMD.

Problem: B=2, N=2048, DIM=1024, 16 query heads / 4 KV heads, head_dim=64, fp32.
Sharding: core c = (batch b=c//4, kv-group g=c%4). Each core computes its
group's 4 query heads + 1 shared KV head over the full sequence, and a partial
output projection (its 256 rows of Wo). Host sums the 4 group partials per
batch and adds the bias.

Data path (all bf16 on PE, fp32 in PSUM):
  - Host pre-transposes x -> xT [1024, 2048] bf16 per batch, slices weights.
  - Projections: Q^T [256(2 head pairs), N], K^T [64, N] (duplicated onto
    both partition halves for head-pair score matmuls), V natural [N, 64]
    stored as vones [128, 16, 65] with an all-ones column for row-sums.
  - Scores S^T [128 keys, q] = K^T-tile (stationary) x Q^T (moving); exp on
    ACT into P [keys, q] bf16 (no max subtraction; |s| small by construction).
  - PV: P-tile [keys, 128 q] stationary x vones [keys, 65] moving ->
    psum [128 q, 65] accumulated over 16 key tiles; col 64 = sum of exp.
  - Normalize with per-partition scalar multiply (DVE), pack A [tok, 256],
    PE-transpose to A^T, out-proj out^T[1024, N] = Wo-tile x A^T.

The q dimension is processed in two mega-chunks of 1024 so exp units are
[128,1024] (2 psum banks, double buffered = 4) + 2 PV banks + 2 outproj/
transpose banks = 8 banks total.
"""

import sys

if "/opt/trn_rl_repo" not in sys.path:
    sys.path.insert(0, "/opt/trn_rl_repo")

from contextlib import ExitStack

import numpy as np

import concourse.bass as bass
import concourse.mybir as mybir
import concourse.tile as tile
from concourse import bacc, bass_utils
from concourse.bass import ds, ts
from concourse.masks import make_identity

F32 = mybir.dt.float32
BF16 = mybir.dt.bfloat16
EXPF = mybir.ActivationFunctionType.Exp

DIM = 1024
D = 64  # head dim
SCALE = D ** -0.5


def build_nc(NSEQ=2048):
    KB = NSEQ // 128          # key tiles
    NC_CH = NSEQ // 512       # 512-token chunks
    MCS = [(0, NSEQ // 2), (NSEQ // 2, NSEQ // 2)]  # (q0, qw) mega-chunks

    nc = bacc.Bacc("TRN2", target_bir_lowering=False, debug=False)
    xT = nc.dram_tensor("xT", [DIM, NSEQ], BF16, kind="ExternalInput").ap()
    wq = nc.dram_tensor("wq", [DIM, 256], BF16, kind="ExternalInput").ap()
    wk = nc.dram_tensor("wk", [DIM, D], BF16, kind="ExternalInput").ap()
    wv = nc.dram_tensor("wv", [DIM, D], BF16, kind="ExternalInput").ap()
    wo = nc.dram_tensor("wo", [256, DIM], BF16, kind="ExternalInput").ap()
    out = nc.dram_tensor("out", [DIM, NSEQ], F32, kind="ExternalOutput").ap()

    with tile.TileContext(nc) as tc, ExitStack() as ctx:
        sb = ctx.enter_context(tc.tile_pool(name="sb", bufs=1))
        dyn = ctx.enter_context(tc.tile_pool(name="dyn", bufs=1))
        ps = ctx.enter_context(tc.tile_pool(name="ps", bufs=1, space="PSUM"))

        # ---- persistent SBUF ----
        wq_sb = sb.tile([128, 8, 256], BF16)
        wk_sb = sb.tile([128, 8, D], BF16)
        wv_sb = sb.tile([128, 8, D], BF16)
        wo_sb = sb.tile([128, 2, DIM], BF16)
        ident = sb.tile([128, 128], BF16)
        xt_sb = sb.tile([128, 8, NSEQ], BF16)
        QT = sb.tile([128, 2, NSEQ], BF16)
        KT = sb.tile([128, NSEQ], BF16)
        vones = sb.tile([128, KB, D + 1], BF16)
        AT = sb.tile([128, 2, NSEQ], BF16)
        warm = sb.tile([128, 1], F32)
        wone = sb.tile([128, 1], BF16)

        # ---- weight / x DMAs (emission order = DMA device order) ----
        # first kb of scores needs x cols 0:1024 + wq + wk as early as
        # possible; stream x in 256-col pieces so Q-proj starts early
        xr = xT.rearrange("(t p) m -> p t m", p=128)
        nc.sync.dma_start(out=xt_sb[:, :, 0:256], in_=xr[:, :, 0:256])
        nc.sync.dma_start(out=wq_sb, in_=wq.rearrange("(t p) m -> p t m", p=128))
        nc.sync.dma_start(out=xt_sb[:, :, 256:512], in_=xr[:, :, 256:512])
        nc.sync.dma_start(out=xt_sb[:, :, 512:768], in_=xr[:, :, 512:768])
        nc.sync.dma_start(out=wk_sb, in_=wk.rearrange("(t p) m -> p t m", p=128))
        nc.sync.dma_start(out=xt_sb[:, :, 768:1024], in_=xr[:, :, 768:1024])
        nc.sync.dma_start(out=wv_sb, in_=wv.rearrange("(t p) m -> p t m", p=128))
        nc.sync.dma_start(out=xt_sb[:, :, 1024:1536], in_=xr[:, :, 1024:1536])
        nc.sync.dma_start(out=wo_sb, in_=wo.rearrange("(t p) m -> p t m", p=128))
        nc.sync.dma_start(out=xt_sb[:, :, 1536:2048], in_=xr[:, :, 1536:2048])
        make_identity(nc, ident)
        nc.vector.memset(vones, 1.0)
        nc.vector.memset(wone, 1.0)
        nc.scalar.activation(out=warm, in_=wone, func=EXPF, scale=1.0)

        # keep PE continuously busy from t~1us so real matmuls start at the
        # ramped clock (p-state) instead of mid speed
        for w in range(24):
            pw = ps.tile([128, 512], BF16, tag="op", bufs=2, name=f"pwarm{w}")
            nc.tensor.transpose(pw[:, 0:128], ident, ident)

        # ---- prologue emitters (128-col pieces so PE blocks stay ~430ns
        # and never starve the ACT exp stream) ----
        def emit_qproj(j, p):
            """Q^T head pair p for 128-token piece j -> QT[:, p, j*128:+128]."""
            pq = ps.tile([128, 1024], F32, tag="sp", bufs=2, name=f"pq{j}_{p}")
            for t in range(8):
                nc.tensor.matmul(pq[:, 0:128], wq_sb[:, t, ts(p, 128)],
                                 xt_sb[:, t, ts(j, 128)],
                                 start=(t == 0), stop=(t == 7))
            nc.vector.tensor_copy(QT[:, p, ts(j, 128)], pq[:, 0:128])

        def emit_kproj(j):
            """K^T for 128-token piece j -> KT[0:64, j*128:+128]."""
            pk = ps.tile([128, 512], F32, tag="pv", bufs=2, name=f"pk{j}")
            for t in range(8):
                nc.tensor.matmul(pk[0:64, 0:128], wk_sb[:, t, :],
                                 xt_sb[:, t, ts(j, 128)],
                                 start=(t == 0), stop=(t == 7))
            nc.vector.tensor_copy(KT[0:64, ts(j, 128)], pk[0:64, 0:128])

        def emit_vproj(tb):
            """V natural for token tile tb -> vones[:, tb, 0:64]."""
            pv = ps.tile([128, 512], F32, tag="op", bufs=2, name=f"pvv{tb}")
            for t in range(8):
                nc.tensor.matmul(pv[:, 0:D],
                                 xt_sb[:, t, ts(tb, 128)], wv_sb[:, t, :],
                                 start=(t == 0), stop=(t == 7))
            nc.vector.tensor_copy(vones[:, tb, 0:D], pv[:, 0:D])

        def emit_ktdup():
            # duplicate K^T onto partitions 64:128 (SBUF->SBUF DMA crosses
            # partitions; engines cannot)
            nc.sync.dma_start(out=KT[64:128, :], in_=KT[0:64, :])

        # ---- attention state ----
        P_of = {}      # window -> P tile
        A_of = {}      # mci -> A tile

        def emit_scores_exp(mci, h, kb):
            q0, qw = MCS[mci]
            p, i = h // 2, h % 2
            sp = ps.tile([128, 1024], F32, tag="sp", bufs=2,
                         name=f"sp{mci}_{h}_{kb}")
            for j in range(qw // 512):
                nc.tensor.matmul(sp[:, ts(j, 512)],
                                 KT[ds(i * D, D), ts(kb, 128)],
                                 QT[ds(i * D, D), p, ds(q0 + j * 512, 512)],
                                 start=True, stop=True)
            nc.scalar.activation(out=P_of[(mci, h)][:, kb, 0:qw],
                                 in_=sp[:, 0:qw], func=EXPF, scale=SCALE)

        def emit_pv(mci, h, qb, tag="pv"):
            """PV for query block qb (128 tokens) of head h, mega-chunk mci;
            includes normalize into A and (for odd heads) the A^T transpose."""
            q0, qw = MCS[mci]
            P = P_of[(mci, h)]
            acc = ps.tile([128, 512] if tag != "sp" else [128, 1024], F32,
                          tag=tag, bufs=2, name=f"acc{mci}_{h}_{qb}")
            for kb in range(KB):
                nc.tensor.matmul(acc[:, 0:D + 1], P[:, kb, ds(qb * 128, 128)],
                                 vones[:, kb, :],
                                 start=(kb == 0), stop=(kb == KB - 1))
            rc = dyn.tile([128, 1], F32, tag="rc", bufs=6,
                          name=f"rc{mci}_{h}_{qb}")
            nc.vector.reciprocal(out=rc, in_=acc[:, D:D + 1])
            nc.vector.tensor_scalar_mul(
                A_of[mci][:, qb, ds(h * D, D)], acc[:, 0:D], rc)
            if h % 2 == 1:
                pr = h // 2
                tp = ps.tile([128, 512], BF16, tag="op", bufs=2,
                             name=f"tp{mci}_{h}_{qb}")
                nc.tensor.transpose(tp[:, 0:128], A_of[mci][:, qb, ts(pr, 128)],
                                    ident)
                nc.vector.tensor_copy(
                    AT[:, pr, ds(q0 + qb * 128, 128)], tp[:, 0:128])

        def emit_outproj(mci, ct, tch, act_copy=False, tag="op"):
            """out^T[ct*128:+128, q0+tch*512:+512]."""
            q0, qw = MCS[mci]
            po = ps.tile([128, 512] if tag != "sp" else [128, 1024], F32,
                         tag=tag, bufs=2, name=f"po{mci}_{ct}_{tch}")
            pv512 = po[:, 0:512]
            for ft in range(2):
                nc.tensor.matmul(pv512, wo_sb[:, ft, ts(ct, 128)],
                                 AT[:, ft, ds(q0 + tch * 512, 512)],
                                 start=(ft == 0), stop=(ft == 1))
            ot = dyn.tile([128, 512], F32, tag="os", bufs=6,
                          name=f"ot{mci}_{ct}_{tch}")
            if act_copy:
                nc.scalar.copy(ot, pv512)
            else:
                nc.vector.tensor_copy(ot, pv512)
            nc.sync.dma_start(out=out[ts(ct, 128), ds(q0 + tch * 512, 512)],
                              in_=ot)

        # ---- window schedule with sprinkled side-work ----
        windows = [(mci, h) for mci in range(2) for h in range(4)]

        # pre-window prologue: enough to start (0, h0): Q pair0 pieces 0..7
        # (cols 0:1024) and K piece 0 (keys 0:128)
        for j in range(4):
            emit_qproj(j, 0)
        emit_kproj(0)
        for j in range(4, 8):
            emit_qproj(j, 0)

        # per-window sprinkle thunks: (cost_ns, fn); kept <=~450ns each so
        # they slot into the PE slack between scores units
        QP = 430
        PV = 450
        OP = 450
        VP = 220

        def sprinkles(wi):
            mci, h = windows[wi]
            th = []
            if wi == 0:
                th += [(QP, lambda j=j: emit_kproj(j)) for j in (1, 2, 3)]
                th += [(QP, lambda j=j: emit_kproj(j)) for j in range(4, 16)]
                th.append((0, emit_ktdup))
                th += [(VP, lambda tb=tb: emit_vproj(tb)) for tb in range(16)]
                th += [(QP, lambda j=j: emit_qproj(j, 1)) for j in range(4)]
            else:
                pmci, ph = windows[wi - 1]
                nqb = MCS[pmci][1] // 128
                th += [(PV, lambda qb=qb, m=pmci, hh=ph: emit_pv(m, hh, qb))
                       for qb in range(nqb)]
                if wi == 1:
                    th += [(QP, lambda j=j: emit_qproj(j, 1))
                           for j in range(4, 8)]
                    th += [(QP, lambda j=j: emit_qproj(j, 0))
                           for j in range(8, 16)]
                elif wi == 2:
                    th += [(QP, lambda j=j: emit_qproj(j, 1))
                           for j in range(8, 16)]
                if wi in (5, 6, 7):  # out-proj of mc0 during (1,h1)..(1,h3)
                    units = [(ct, tch) for tch in range(2) for ct in range(8)]
                    part = units[(wi - 5) * 6:(wi - 4) * 6]
                    th += [(OP, lambda u=u: emit_outproj(0, u[0], u[1]))
                           for u in part]
            return th

        for wi, (mci, h) in enumerate(windows):
            P_of[(mci, h)] = dyn.tile([128, KB, 1024], BF16, tag="P", bufs=2,
                                      name=f"P{mci}_{h}")
            if h == 1:
                A_of[mci] = dyn.tile([128, MCS[mci][1] // 128, 256], BF16,
                                     tag="A", bufs=2, name=f"A{mci}")
            th = sprinkles(wi)
            total = sum(c for c, _ in th)
            spent = 0
            for kb in range(KB):
                emit_scores_exp(mci, h, kb)
                # spread side work across the window, ~even by cost
                goal = total * (kb + 1) // KB
                while th and spent < goal:
                    c, f = th.pop(0)
                    f()
                    spent += c
            while th:
                th.pop(0)[1]()

        # ---- tail: PV of last head (4 psum lanes: pv,pv,sp,sp) then
        # out-proj of mc1 interleaved across op/sp psum and ACT/DVE copies
        for qb in range(4):
            emit_pv(1, 3, qb, tag=("pv", "pv", "sp", "sp")[qb])
        for i, ct in enumerate(range(4)):
            emit_outproj(1, ct, 0, act_copy=i % 2 == 0,
                         tag="op" if i % 2 == 0 else "sp")
            emit_pv(1, 3, 4 + i, tag="pv" if i % 2 == 0 else "sp")
        for i, ct in enumerate(range(4, 8)):
            emit_outproj(1, ct, 0, act_copy=i % 2 == 0,
                         tag="op" if i % 2 == 0 else "sp")
        for i, ct in enumerate(range(8)):
            emit_outproj(1, ct, 1, act_copy=i % 2 == 0,
                         tag="op" if i % 2 == 0 else "sp")

    nc.compile()
    return nc


_CACHE = {}


def _get_nc(NSEQ):
    if NSEQ not in _CACHE:
        _CACHE[NSEQ] = build_nc(NSEQ)
    return _CACHE[NSEQ]


def kernel(x, Wq, Wk, Wv, Wo, bo):
    """Full-input entry point: shard over 8 cores, run, gather."""
    import ml_dtypes
    bf16 = ml_dtypes.bfloat16
    x = np.asarray(x, np.float32)
    Wq = np.asarray(Wq, np.float32)
    Wk = np.asarray(Wk, np.float32)
    Wv = np.asarray(Wv, np.float32)
    Wo = np.asarray(Wo, np.float32)
    bo = np.asarray(bo, np.float32)
    B, N, C = x.shape
    nc = _get_nc(N)
    in_maps = []
    for c in range(8):
        b, g = c // 4, c % 4
        in_maps.append({
            "xT": np.ascontiguousarray(x[b].T).astype(bf16),
            "wq": np.ascontiguousarray(Wq[:, g * 256:(g + 1) * 256]).astype(bf16),
            "wk": np.ascontiguousarray(Wk[:, g * D:(g + 1) * D]).astype(bf16),
            "wv": np.ascontiguousarray(Wv[:, g * D:(g + 1) * D]).astype(bf16),
            "wo": np.ascontiguousarray(Wo[g * 256:(g + 1) * 256, :]).astype(bf16),
        })
    res = bass_utils.run_bass_kernel_spmd(nc, in_maps, core_ids=list(range(8)))
    outs = [res.results[c]["out"] for c in range(8)]
    full = np.empty((B, N, C), np.float32)
    for b in range(B):
        acc = outs[4 * b].astype(np.float32)
        for g in range(1, 4):
            acc = acc + outs[4 * b + g]
        full[b] = acc.T + bo[None, :]
    return full
